# revision 27
# baseline (speedup 1.0000x reference)
"""Trainium2 Bass kernel for nn_DeformableSliceGrouped.

Sharding: 8 cores = 2 batches x 4 h-quarters. Each core handles
(1, 256, 32, 12, 48) of the input in a (c, hw, z) "z-innermost" layout.

Host folds the positional encoding into the GEMM input (xpe16 = x + pe),
so the device pipeline is:

  W:   W2 = o_w@v_w (fp32r weight prep)
  P1a: q GEMMs over xpe16 -> per-z running max (qred on gpsimd)
  B1:  8-core AllReduce-max of qp (both batches in 2 slots) -> syncs all
       cores; offsets/attn/softmax -> mixing matrix M -> block-diag w128
  P1b: v' = W2 @ xpe16 in vT layout (hides B1); copies on scalar
  P2a: mix via PE (z-on-partitions block-diag M.T); copy h0 scalar /
       h1 gpsimd; bn_stats on f16 vbuf (vector). x16 streams in.
  B2:  8-core AllReduce-add BN partial stats (~20us, skew-free after B1)
  P2b: normalize + residual + store, 4-tile chunks, split across engines
"""
from contextlib import ExitStack

import numpy as np

import concourse.bass as bass
import concourse.bacc as bacc
import concourse.tile as tile
import concourse.mybir as mybir
from concourse import bass_utils

F32 = mybir.dt.float32
F32R = mybir.dt.float32r
F16 = mybir.dt.float16

B, C, Z, H, W = 2, 256, 32, 48, 48
HL = H // 4            # h rows per core
FL = HL * W            # 576 hw positions per core
FPT = 16               # hw positions per tile
G = FL // FPT          # 36 tiles
CH = 4                 # tiles per P2b chunk
NCH = G // CH          # 9 chunks
NHP = 6                # heads*points
NLOC = FL * Z          # per-core elements per channel (18432)
NTOT = 8 * NLOC        # global elements per channel (147456)
EPS = 1e-5
NEG = -60000.0  # f16-safe "minus infinity" for the max-reduce slots

ALL_GROUP = [[0, 1, 2, 3, 4, 5, 6, 7]]


def ts(i, n):
    return slice(i * n, (i + 1) * n)


def _emit(tc, t):
    nc = tc.nc
    ctx = ExitStack()
    cp = ctx.enter_context(tc.tile_pool(name="consts", bufs=1))
    xpe_pool = ctx.enter_context(tc.tile_pool(name="xpe", bufs=1))
    bigp = ctx.enter_context(tc.tile_pool(name="big", bufs=1))
    resp = ctx.enter_context(tc.tile_pool(name="res", bufs=3))
    dramp = ctx.enter_context(tc.tile_pool(name="dram", bufs=1, space="DRAM"))

    # ---- constant loads (sync queue): qwT + first x chunks first ----
    qwT = cp.tile([128, 2, C], F16)
    nc.sync.dma_start(qwT[:], t["qwT16"].rearrange("(h p) m -> p h m", p=128))

    # xpe16 streamed in big chunks; first two right behind qwT
    x_all = xpe_pool.tile([128, 2, G, FPT * Z], F16, tag="xbig")
    xsrc = t["xpe16"].rearrange("(h p) f z -> p h f z", p=128)

    def load_xpe(j):
        nc.sync.dma_start(
            x_all[:, :, ts(j, CH), :].rearrange("p h g (f z) -> p h (g f) z", z=Z),
            xsrc[:, :, ts(j, CH * FPT), :],
        )

    load_xpe(0)
    load_xpe(1)
    sawT = cp.tile([128, 2, 12], F16)
    nc.sync.dma_start(sawT[:], t["sawT16"].rearrange("(h p) j -> p h j", p=128))
    ident = cp.tile([Z, Z], F16)
    nc.sync.dma_start(ident[:], t["ident"][:])
    iota = cp.tile([Z, Z], F32)
    nc.sync.dma_start(iota[:], t["iota"][:])
    sab = cp.tile([Z, 12], F32)
    sab_src = bass.AP(tensor=t["sab"].tensor, offset=0, ap=[[0, Z], [1, 12]])
    nc.sync.dma_start(sab[:], sab_src)
    gam = cp.tile([128, 2, 1], F32)
    nc.sync.dma_start(gam[:], t["gb"].rearrange("(h p) j -> p h j", p=128)[:, :, 0:1])
    bet = cp.tile([128, 2, 1], F32)
    nc.sync.dma_start(bet[:], t["gb"].rearrange("(h p) j -> p h j", p=128)[:, :, 1:2])
    bslc = cp.tile([128, 2], F32)
    nc.sync.dma_start(bslc[:], t["bsl"][:])
    id128 = cp.tile([128, 128], F16)
    nc.sync.dma_start(id128[:], t["id128"][:])
    iot128 = cp.tile([128, 128], F16)
    nc.sync.dma_start(iot128[:], t["iot128"][:])
    pid = cp.tile([128, 1], F32)
    nc.sync.dma_start(pid[:], t["pid"][:])
    # weight-prep inputs on gpsimd queue
    vw_r = cp.tile([128, 2, C], F32R)
    nc.gpsimd.dma_start(vw_r[:], t["vw"].rearrange("(h p) m -> p h m", p=128))
    owT_r = cp.tile([128, 2, C], F32R)
    nc.gpsimd.dma_start(owT_r[:], t["owT"].rearrange("(h p) m -> p h m", p=128))

    for j in range(2, NCH):
        load_xpe(j)

    vbuf = bigp.tile([128, 2, G, FPT * Z], F16)

    # ---- W phase: W2 = o_w @ v_w ----
    with tc.tile_pool(name="pw", bufs=1, space="PSUM") as pw:
        w2ps = pw.tile([128, 2, C], F32, tag="w2t")
        for mh in range(2):
            for kh in range(2):
                nc.tensor.matmul(
                    w2ps[:, mh, :], vw_r[:, kh, ts(mh, 128)], owT_r[:, kh, :],
                    start=(kh == 0), stop=(kh == 1),
                )
        w2T = cp.tile([128, 2, C], F16)
        nc.scalar.copy(w2T[:], w2ps[:])

    qmax = cp.tile([128, 2, Z], F32)
    nc.vector.memset(qmax[:], NEG)
    w128 = cp.tile([128, 128], F16)
    nc.vector.memset(w128[:], 0.0)
    stats = cp.tile([128, 2, G, 6], F32)

    # ---- P1a: q GEMMs -> per-z max on gpsimd ----
    with tc.tile_pool(name="p1q", bufs=3, space="PSUM") as p1q:
        for g in range(G):
            qps = p1q.tile([128, 2, FPT * Z], F32, tag="qps")
            for mh in range(2):
                for kh in range(2):
                    nc.tensor.matmul(
                        qps[:, mh, :], qwT[:, kh, ts(mh, 128)], x_all[:, kh, g, :],
                        start=(kh == 0), stop=(kh == 1),
                    )
            qred = cp.tile([128, 2, Z], F32, tag=f"qred{g % 3}")
            nc.vector.tensor_reduce(
                out=qred[:],
                in_=qps[:].rearrange("p h (f z) -> p h z f", z=Z),
                axis=mybir.AxisListType.X,
                op=mybir.AluOpType.max,
            )
            nc.vector.tensor_max(qmax[:], qmax[:], qred[:])

    # ---- B1: 8-core qp allreduce-max (both batch slots, f16 payload) ----
    qpin_sb = cp.tile([128, 2, 2, Z], F16)
    for s in range(2):
        nc.vector.tensor_scalar(
            out=qpin_sb[:, :, s, :], in0=qmax[:], scalar1=bslc[:, s:s + 1],
            scalar2=None, op0=mybir.AluOpType.add,
        )
    qpin = dramp.tile([128, 2, 2, Z], F16)
    qpout = dramp.tile([128, 2, 2, Z], F16)
    nc.sync.dma_start(qpin[:], qpin_sb[:])
    nc.gpsimd.collective_compute(
        "AllReduce", mybir.AluOpType.max, replica_groups=ALL_GROUP,
        ins=[qpin[:].opt()], outs=[qpout[:].opt()],
    )
    qpout_sb = cp.tile([128, 2, 2, Z], F16)
    nc.gpsimd.dma_start(qpout_sb[:], qpout[:])
    qsel0 = cp.tile([128, 2, Z], F32)
    nc.vector.tensor_scalar(
        out=qsel0[:], in0=qpout_sb[:, :, 0, :], scalar1=bslc[:, 0:1],
        scalar2=None, op0=mybir.AluOpType.add,
    )
    qsel1 = cp.tile([128, 2, Z], F32)
    nc.vector.tensor_scalar(
        out=qsel1[:], in0=qpout_sb[:, :, 1, :], scalar1=bslc[:, 1:2],
        scalar2=None, op0=mybir.AluOpType.add,
    )
    qp16 = cp.tile([128, 2, Z], F16)
    nc.vector.tensor_tensor(
        out=qp16[:], in0=qsel0[:], in1=qsel1[:], op=mybir.AluOpType.max
    )

    # ---- P1b: v' GEMMs (hide B1). vT layout: pos on partitions ----
    psctx = ExitStack()
    p1w = psctx.enter_context(tc.tile_pool(name="p1w", bufs=2, space="PSUM"))
    pmix = psctx.enter_context(tc.tile_pool(name="pmix", bufs=2, space="PSUM"))

    def emit_vprime(g):
        for pair in range(2):
            vps = p1w.tile([128, 2, C], F32, tag="vps")
            for i in range(2):
                blk = pair * 2 + i
                for kh in range(2):
                    nc.tensor.matmul(
                        vps[:, i, :], x_all[:, kh, g, ts(blk, 128)],
                        w2T[:, kh, :], start=(kh == 0), stop=(kh == 1),
                        skip_group_check=True,
                    )
            out_view = vbuf[:, :, g, ts(pair, 2 * 128)].rearrange(
                "p h (b c) -> p b h c", b=2
            )
            in_view = vps[:].rearrange("p b (h c) -> p b h c", h=2)
            nc.scalar.copy(out_view, in_view)

    def emit_mix(g):
        mix = pmix.tile([128, 2, FPT * Z], F32, tag="mix")
        for h in range(2):
            for blk in range(4):
                nc.tensor.matmul(
                    mix[:, h, ts(blk, 128)], vbuf[:, h, g, ts(blk, 128)],
                    w128[:], start=True, stop=True,
                )
        nc.scalar.copy(vbuf[:, :, g, :], mix[:])
        for h in range(2):
            nc.vector.bn_stats(stats[:, h, g, :], vbuf[:, h, g, :])

    VSPLIT = 26
    for g in range(VSPLIT):
        emit_vprime(g)

    # ---- B1-dependent: offsets/attn -> M -> w128 (vector + psml) ----
    psml = psctx.enter_context(tc.tile_pool(name="psml", bufs=2, space="PSUM"))
    sa_ps = psml.tile([Z, 12], F32, tag="small")
    for kh in range(2):
        nc.tensor.matmul(
            sa_ps[:], qp16[:, kh, :], sawT[:, kh, :],
            start=(kh == 0), stop=(kh == 1),
        )
    logits = cp.tile([Z, 12], F32)
    nc.vector.tensor_add(logits[:], sa_ps[:], sab[:])
    off = cp.tile([Z, NHP], F32)
    nc.vector.tensor_scalar(
        out=off[:], in0=logits[:, 0:NHP], scalar1=0.0, scalar2=float(Z - 1),
        op0=mybir.AluOpType.max, op1=mybir.AluOpType.min,
    )
    ex = cp.tile([Z, NHP], F32)
    nc.scalar.activation(ex[:], logits[:, NHP:12], mybir.ActivationFunctionType.Exp)
    ssum = cp.tile([Z, 1], F32)
    nc.vector.tensor_reduce(
        out=ssum[:], in_=ex[:], axis=mybir.AxisListType.X, op=mybir.AluOpType.add
    )
    rinv = cp.tile([Z, 1], F32)
    nc.vector.reciprocal(rinv[:], ssum[:])
    attn = cp.tile([Z, NHP], F32)
    nc.vector.tensor_scalar_mul(attn[:], ex[:], rinv[:, 0:1])

    # M[z, y] = sum_p attn[z,p] * relu(1 - |off[z,p] - y|)
    msb = cp.tile([Z, Z], F32)
    mtmp = cp.tile([Z, Z], F32)
    dmy = cp.tile([Z, Z], F32)
    um = cp.tile([Z, Z], F32)
    vm = cp.tile([Z, Z], F32)
    for p in range(NHP):
        nc.vector.tensor_scalar(
            out=dmy[:], in0=iota[:], scalar1=off[:, p:p + 1], scalar2=None,
            op0=mybir.AluOpType.subtract,
        )
        nc.vector.tensor_scalar(
            out=um[:], in0=dmy[:], scalar1=-1.0, scalar2=1.0,
            op0=mybir.AluOpType.mult, op1=mybir.AluOpType.add,
        )
        nc.vector.tensor_scalar_add(vm[:], dmy[:], 1.0)
        nc.vector.tensor_tensor(
            out=um[:], in0=um[:], in1=vm[:], op=mybir.AluOpType.min
        )
        dst = msb if p == 0 else mtmp
        nc.vector.tensor_scalar(
            out=dst[:], in0=um[:], scalar1=0.0, scalar2=attn[:, p:p + 1],
            op0=mybir.AluOpType.max, op1=mybir.AluOpType.mult,
        )
        if p > 0:
            nc.vector.tensor_add(msb[:], msb[:], mtmp[:])
    m16 = cp.tile([Z, Z], F16)
    nc.vector.tensor_copy(m16[:], msb[:])
    mt_ps = psml.tile([Z, Z], F16, tag="small")
    nc.tensor.transpose(mt_ps[:], m16[:], ident[:])
    mt16 = cp.tile([Z, Z], F16)
    nc.vector.tensor_copy(mt16[:], mt_ps[:])
    for b4 in range(4):
        nc.vector.tensor_copy(w128[ts(b4, 32), ts(b4, 32)], mt16[:])

    # ---- P2a: remaining v' interleaved with mix; then mix tail ----
    for g in range(VSPLIT, G):
        emit_vprime(g)
        emit_mix(g - VSPLIT)
    for g in range(G - VSPLIT, G):
        emit_mix(g)

    # x16 (residual) reuses the buffer freed by xpe16 (same pool/tag)
    x16_all = xpe_pool.tile([128, 2, G, FPT * Z], F16, tag="xbig")
    rsrc = t["x16"].rearrange("(h p) f z -> p h f z", p=128)
    for j in range(NCH):
        nc.sync.dma_start(
            x16_all[:, :, ts(j, CH), :].rearrange("p h g (f z) -> p h (g f) z", z=Z),
            rsrc[:, :, ts(j, CH * FPT), :],
        )

    # ---- B2: global BN stats (8-core add) ----
    mv = cp.tile([128, 2, 2], F32)
    for h in range(2):
        nc.vector.bn_aggr(mv[:, h, :], stats[:, h, :, :])
    msq = cp.tile([128, 2, 1], F32)
    nc.vector.tensor_mul(msq[:], mv[:, :, 0:1], mv[:, :, 0:1])
    ex2 = cp.tile([128, 2, 1], F32)
    nc.vector.tensor_add(ex2[:], mv[:, :, 1:2], msq[:])
    red_in = cp.tile([128, 2, 2], F32)
    nc.vector.tensor_scalar_mul(red_in[:, :, 0:1], mv[:, :, 0:1], float(NLOC))
    nc.vector.tensor_scalar_mul(red_in[:, :, 1:2], ex2[:], float(NLOC))
    rin = dramp.tile([128, 2, 2], F32)
    rout = dramp.tile([128, 2, 2], F32)
    nc.sync.dma_start(rin[:], red_in[:])
    nc.gpsimd.collective_compute(
        "AllReduce", mybir.AluOpType.add, replica_groups=ALL_GROUP,
        ins=[rin[:].opt()], outs=[rout[:].opt()],
    )
    gst = cp.tile([128, 2, 2], F32)
    nc.gpsimd.dma_start(gst[:], rout[:])
    mean_g = cp.tile([128, 2, 1], F32)
    nc.vector.tensor_scalar_mul(mean_g[:], gst[:, :, 0:1], 1.0 / NTOT)
    ex2g = cp.tile([128, 2, 1], F32)
    nc.vector.tensor_scalar_mul(ex2g[:], gst[:, :, 1:2], 1.0 / NTOT)
    m2g = cp.tile([128, 2, 1], F32)
    nc.vector.tensor_mul(m2g[:], mean_g[:], mean_g[:])
    var_g = cp.tile([128, 2, 1], F32)
    nc.vector.tensor_sub(var_g[:], ex2g[:], m2g[:])
    eps_sb = cp.tile([128, 1], F32)
    nc.vector.memset(eps_sb[:], EPS)
    sd = cp.tile([128, 2, 1], F32)
    nc.scalar.activation(
        sd[:], var_g[:], mybir.ActivationFunctionType.Sqrt, bias=eps_sb[:]
    )
    rs = cp.tile([128, 2, 1], F32)
    nc.vector.reciprocal(rs[:], sd[:])
    s_sb = cp.tile([128, 2, 1], F32)
    nc.vector.tensor_mul(s_sb[:], rs[:], gam[:])
    ms_t = cp.tile([128, 2, 1], F32)
    nc.vector.tensor_mul(ms_t[:], mean_g[:], s_sb[:])
    t_sb = cp.tile([128, 2, 1], F32)
    nc.vector.tensor_sub(t_sb[:], bet[:], ms_t[:])

    # diag(s0) for the PE-side h0 normalize: mask[p,j] = (j == p), row-scaled
    dmask = cp.tile([128, 128], F16)
    nc.vector.tensor_scalar(
        out=dmask[:], in0=iot128[:], scalar1=pid[:, 0:1], scalar2=None,
        op0=mybir.AluOpType.is_equal,
    )
    diag0 = cp.tile([128, 128], F16)
    nc.vector.tensor_scalar_mul(diag0[:], dmask[:], s_sb[:, 0, :])

    psctx.close()  # free p1w/pmix/psml banks for the P2b psum pool

    # ---- P2b: normalize + residual + store in 4-tile chunks.
    # h0: PE computes s0*mix + x16 (diag + identity matmuls); scalar adds
    # t0 in the PSUM->SBUF copy. h1: vector/gpsimd elementwise path.
    pres = ctx.enter_context(tc.tile_pool(name="pres", bufs=2, space="PSUM"))
    out_v = t["out"].rearrange("(h p) f z -> p h f z", p=128)
    CF = CH * FPT * Z
    for j in range(NCH):
        psum_r = pres.tile([128, CH, FPT * Z], F32, tag="pr")
        for i in range(CH):
            nc.tensor.matmul(
                psum_r[:, i, :], diag0[:], vbuf[:, 0, j * CH + i, :],
                start=True, stop=False, skip_group_check=True,
            )
            nc.tensor.matmul(
                psum_r[:, i, :], id128[:], x16_all[:, 0, j * CH + i, :],
                start=False, stop=True, skip_group_check=True,
            )
        res0 = resp.tile([128, CH, FPT * Z], F16, tag="res0")
        nc.scalar.activation(
            res0[:], psum_r[:],
            mybir.ActivationFunctionType.Identity, bias=t_sb[:, 0, :],
        )
        nrm1 = resp.tile([128, CF], F16, tag="nrm1")
        nc.vector.tensor_scalar(
            out=nrm1[:], in0=vbuf[:, 1, ts(j, CH), :],
            scalar1=s_sb[:, 1, :], scalar2=t_sb[:, 1, :],
            op0=mybir.AluOpType.mult, op1=mybir.AluOpType.add,
        )
        res1 = resp.tile([128, CF], F16, tag="res1")
        nc.vector.tensor_add(
            res1[:, 0:CF // 2], nrm1[:, 0:CF // 2],
            x16_all[:, 1, ts(j, CH), :].rearrange("p a b -> p (a b)")[:, 0:CF // 2],
        )
        nc.gpsimd.tensor_add(
            res1[:, CF // 2:], nrm1[:, CF // 2:],
            x16_all[:, 1, ts(j, CH), :].rearrange("p a b -> p (a b)")[:, CF // 2:],
        )
        nc.sync.dma_start(
            out_v[:, 0, ts(j, CH * FPT), :],
            res0[:].rearrange("p g (f z) -> p (g f) z", z=Z),
        )
        nc.sync.dma_start(
            out_v[:, 1, ts(j, CH * FPT), :],
            res1[:].rearrange("p (f z) -> p f z", z=Z),
        )
    ctx.close()


_BUILT = None


def _build():
    global _BUILT
    if _BUILT is not None:
        return _BUILT
    nc = bacc.Bacc("TRN2", target_bir_lowering=False, debug=False, num_devices=8)
    t = {}
    t["xpe16"] = nc.dram_tensor("xpe16", [C, FL, Z], F16, kind="ExternalInput").ap()
    t["x16"] = nc.dram_tensor("x16", [C, FL, Z], F16, kind="ExternalInput").ap()
    t["qwT16"] = nc.dram_tensor("qwT16", [C, C], F16, kind="ExternalInput").ap()
    t["vw"] = nc.dram_tensor("vw", [C, C], F32, kind="ExternalInput").ap()
    t["owT"] = nc.dram_tensor("owT", [C, C], F32, kind="ExternalInput").ap()
    t["sawT16"] = nc.dram_tensor("sawT16", [C, 12], F16, kind="ExternalInput").ap()
    t["sab"] = nc.dram_tensor("sab", [12], F32, kind="ExternalInput").ap()
    t["iota"] = nc.dram_tensor("iota", [Z, Z], F32, kind="ExternalInput").ap()
    t["ident"] = nc.dram_tensor("ident", [Z, Z], F16, kind="ExternalInput").ap()
    t["gb"] = nc.dram_tensor("gb", [C, 2], F32, kind="ExternalInput").ap()
    t["bsl"] = nc.dram_tensor("bsl", [128, 2], F32, kind="ExternalInput").ap()
    t["id128"] = nc.dram_tensor("id128", [128, 128], F16, kind="ExternalInput").ap()
    t["iot128"] = nc.dram_tensor("iot128", [128, 128], F16, kind="ExternalInput").ap()
    t["pid"] = nc.dram_tensor("pid", [128, 1], F32, kind="ExternalInput").ap()
    t["out"] = nc.dram_tensor("out", [C, FL, Z], F16, kind="ExternalOutput").ap()
    with tile.TileContext(nc) as tc:
        _emit(tc, t)
    nc.compile()
    _BUILT = nc
    return nc


def _make_pe():
    pos = np.arange(Z, dtype=np.float32)[:, None]
    div = np.exp(np.arange(0, C, 2, dtype=np.float32) * (-np.log(10000.0) / C))
    pe = np.zeros((Z, C), dtype=np.float32)
    pe[:, 0::2] = np.sin(pos * div)
    pe[:, 1::2] = np.cos(pos * div)
    return pe


def _prepare_in_maps(features, q_w, v_w, o_w, offs_w, offs_b, attn_w, attn_b,
                     gamma, beta):
    features = np.ascontiguousarray(np.asarray(features, dtype=np.float32))
    pe = _make_pe()  # (Z, C)
    shared = {
        "qwT16": np.ascontiguousarray(np.asarray(q_w).T).astype(np.float16),
        "vw": np.ascontiguousarray(np.asarray(v_w, dtype=np.float32)),
        "owT": np.ascontiguousarray(np.asarray(o_w).T.astype(np.float32)),
        "sawT16": np.concatenate(
            [np.asarray(offs_w).T, np.asarray(attn_w).T], axis=1
        ).astype(np.float16),
        "sab": np.concatenate(
            [np.asarray(offs_b), np.asarray(attn_b)]
        ).astype(np.float32),
        "iota": np.tile(np.arange(Z, dtype=np.float32)[None, :], (Z, 1)),
        "ident": np.eye(Z, dtype=np.float16),
        "id128": np.eye(128, dtype=np.float16),
        "iot128": np.tile(np.arange(128, dtype=np.float16)[None, :], (128, 1)),
        "pid": np.arange(128, dtype=np.float32)[:, None],
        "gb": np.stack(
            [np.asarray(gamma, np.float32), np.asarray(beta, np.float32)], axis=1
        ),
    }
    in_maps = []
    for k in range(8):
        bi, hq = k // 4, k % 4
        xs = features[bi][:, :, hq * HL:(hq + 1) * HL, :]
        xs = np.ascontiguousarray(xs.transpose(0, 2, 3, 1)).reshape(C, FL, Z)
        xpe = xs + pe.T[:, None, :]  # pe[z, c] -> (C, 1, Z) broadcast
        m = dict(shared)
        m["x16"] = xs.astype(np.float16)
        m["xpe16"] = xpe.astype(np.float16)
        bsl = np.zeros((128, 2), np.float32)
        bsl[:, 1 - bi] = NEG
        m["bsl"] = bsl
        in_maps.append(m)
    return in_maps


def kernel(**inputs):
    nc = _build()
    in_maps = _prepare_in_maps(**inputs)
    res = bass_utils.run_bass_kernel_spmd(nc, in_maps, core_ids=list(range(8)))

    full = np.empty((B, C, Z, H, W), dtype=np.float32)
    for k in range(8):
        bi, hq = k // 4, k % 4
        o = res.results[k]["out"].astype(np.float32).reshape(C, HL, W, Z).transpose(0, 3, 1, 2)
        full[bi][:, :, hq * HL:(hq + 1) * HL, :] = o
    return full


# revision 32
# speedup vs baseline: 1.0617x; 1.0617x over previous
"""Trainium2 Bass kernel for nn_DeformableSliceGrouped.

Sharding: 8 cores = 2 batches x 4 h-quarters. Each core handles
(1, 256, 32, 12, 48) of the input in a (c, hw, z) "z-innermost" layout.

Host folds the positional encoding into the GEMM input (xpe16 = x + pe),
so the device pipeline is:

  W:   W2 = o_w@v_w (fp32r weight prep)
  P1a: q GEMMs over xpe16 -> per-z running max (qred on gpsimd)
  B1:  8-core AllReduce-max of qp (both batches in 2 slots) -> syncs all
       cores; offsets/attn/softmax -> mixing matrix M -> block-diag w128
  P1b: v' = W2 @ xpe16 in vT layout (hides B1); copies on scalar
  P2a: mix via PE (z-on-partitions block-diag M.T); copy h0 scalar /
       h1 gpsimd; bn_stats on f16 vbuf (vector). x16 streams in.
  B2:  8-core AllReduce-add BN partial stats (~20us, skew-free after B1)
  P2b: normalize + residual + store, 4-tile chunks, split across engines
"""
from contextlib import ExitStack

import numpy as np

import concourse.bass as bass
import concourse.bacc as bacc
import concourse.tile as tile
import concourse.mybir as mybir
from concourse import bass_utils

F32 = mybir.dt.float32
F32R = mybir.dt.float32r
F16 = mybir.dt.float16

B, C, Z, H, W = 2, 256, 32, 48, 48
HL = H // 4            # h rows per core
FL = HL * W            # 576 hw positions per core
FPT = 16               # hw positions per tile
G = FL // FPT          # 36 tiles
CH = 4                 # tiles per P2b chunk
NCH = G // CH          # 9 chunks
NHP = 6                # heads*points
NLOC = FL * Z          # per-core elements per channel (18432)
NTOT = 8 * NLOC        # global elements per channel (147456)
EPS = 1e-5
NEG = -60000.0  # f16-safe "minus infinity" for the max-reduce slots

ALL_GROUP = [[0, 1, 2, 3, 4, 5, 6, 7]]


def ts(i, n):
    return slice(i * n, (i + 1) * n)


def _emit(tc, t):
    nc = tc.nc
    ctx = ExitStack()
    cp = ctx.enter_context(tc.tile_pool(name="consts", bufs=1))
    xpe_pool = ctx.enter_context(tc.tile_pool(name="xpe", bufs=1))
    bigp = ctx.enter_context(tc.tile_pool(name="big", bufs=1))
    resp = ctx.enter_context(tc.tile_pool(name="res", bufs=3))
    dramp = ctx.enter_context(tc.tile_pool(name="dram", bufs=1, space="DRAM"))

    # ---- constant loads (sync queue): qwT + first x chunks first ----
    qwT = cp.tile([128, 2, C], F16)
    nc.sync.dma_start(qwT[:], t["qwT16"].rearrange("(h p) m -> p h m", p=128))

    # xpe16 streamed in big chunks; first two right behind qwT
    x_all = xpe_pool.tile([128, 2, G, FPT * Z], F16, tag="xbig")
    xsrc = t["xpe16"].rearrange("(h p) f z -> p h f z", p=128)

    def load_xpe(j):
        nc.sync.dma_start(
            x_all[:, :, ts(j, CH), :].rearrange("p h g (f z) -> p h (g f) z", z=Z),
            xsrc[:, :, ts(j, CH * FPT), :],
        )

    load_xpe(0)
    load_xpe(1)
    sawT = cp.tile([128, 2, 12], F16)
    nc.sync.dma_start(sawT[:], t["sawT16"].rearrange("(h p) j -> p h j", p=128))
    ident = cp.tile([Z, Z], F16)
    nc.sync.dma_start(ident[:], t["ident"][:])
    iota = cp.tile([Z, Z], F32)
    nc.sync.dma_start(iota[:], t["iota"][:])
    sab = cp.tile([Z, 12], F32)
    sab_src = bass.AP(tensor=t["sab"].tensor, offset=0, ap=[[0, Z], [1, 12]])
    nc.sync.dma_start(sab[:], sab_src)
    gam = cp.tile([128, 2, 1], F32)
    nc.sync.dma_start(gam[:], t["gb"].rearrange("(h p) j -> p h j", p=128)[:, :, 0:1])
    bet = cp.tile([128, 2, 1], F32)
    nc.sync.dma_start(bet[:], t["gb"].rearrange("(h p) j -> p h j", p=128)[:, :, 1:2])
    bslc = cp.tile([128, 2], F32)
    nc.sync.dma_start(bslc[:], t["bsl"][:])

    # weight-prep inputs on gpsimd queue
    vw_r = cp.tile([128, 2, C], F32R)
    nc.gpsimd.dma_start(vw_r[:], t["vw"].rearrange("(h p) m -> p h m", p=128))
    owT_r = cp.tile([128, 2, C], F32R)
    nc.gpsimd.dma_start(owT_r[:], t["owT"].rearrange("(h p) m -> p h m", p=128))

    for j in range(2, NCH):
        load_xpe(j)

    vbuf = bigp.tile([128, 2, G, FPT * Z], F16)

    # ---- W phase: W2 = o_w @ v_w ----
    with tc.tile_pool(name="pw", bufs=1, space="PSUM") as pw:
        w2ps = pw.tile([128, 2, C], F32, tag="w2t")
        for mh in range(2):
            for kh in range(2):
                nc.tensor.matmul(
                    w2ps[:, mh, :], vw_r[:, kh, ts(mh, 128)], owT_r[:, kh, :],
                    start=(kh == 0), stop=(kh == 1),
                )
        w2T = cp.tile([128, 2, C], F16)
        nc.scalar.copy(w2T[:], w2ps[:])

    qmax = cp.tile([128, 2, Z], F32)
    nc.vector.memset(qmax[:], NEG)
    w128 = cp.tile([128, 128], F16)
    nc.vector.memset(w128[:], 0.0)
    stats = cp.tile([128, 2, G, 6], F32)

    # ---- P1a: q GEMMs -> per-z max on gpsimd ----
    with tc.tile_pool(name="p1q", bufs=3, space="PSUM") as p1q:
        for g in range(G):
            qps = p1q.tile([128, 2, FPT * Z], F32, tag="qps")
            for mh in range(2):
                for kh in range(2):
                    nc.tensor.matmul(
                        qps[:, mh, :], qwT[:, kh, ts(mh, 128)], x_all[:, kh, g, :],
                        start=(kh == 0), stop=(kh == 1),
                    )
            qred = cp.tile([128, 2, Z], F32, tag=f"qred{g % 3}")
            nc.vector.tensor_reduce(
                out=qred[:],
                in_=qps[:].rearrange("p h (f z) -> p h z f", z=Z),
                axis=mybir.AxisListType.X,
                op=mybir.AluOpType.max,
            )
            nc.vector.tensor_max(qmax[:], qmax[:], qred[:])

    # ---- B1: 8-core qp allreduce-max (both batch slots, f16 payload) ----
    qpin_sb = cp.tile([128, 2, 2, Z], F16)
    for s in range(2):
        nc.vector.tensor_scalar(
            out=qpin_sb[:, :, s, :], in0=qmax[:], scalar1=bslc[:, s:s + 1],
            scalar2=None, op0=mybir.AluOpType.add,
        )
    qpin = dramp.tile([128, 2, 2, Z], F16)
    qpout = dramp.tile([128, 2, 2, Z], F16)
    nc.sync.dma_start(qpin[:], qpin_sb[:])
    nc.gpsimd.collective_compute(
        "AllReduce", mybir.AluOpType.max, replica_groups=ALL_GROUP,
        ins=[qpin[:].opt()], outs=[qpout[:].opt()],
    )
    qpout_sb = cp.tile([128, 2, 2, Z], F16)
    nc.gpsimd.dma_start(qpout_sb[:], qpout[:])
    qsel0 = cp.tile([128, 2, Z], F32)
    nc.vector.tensor_scalar(
        out=qsel0[:], in0=qpout_sb[:, :, 0, :], scalar1=bslc[:, 0:1],
        scalar2=None, op0=mybir.AluOpType.add,
    )
    qsel1 = cp.tile([128, 2, Z], F32)
    nc.vector.tensor_scalar(
        out=qsel1[:], in0=qpout_sb[:, :, 1, :], scalar1=bslc[:, 1:2],
        scalar2=None, op0=mybir.AluOpType.add,
    )
    qp16 = cp.tile([128, 2, Z], F16)
    nc.vector.tensor_tensor(
        out=qp16[:], in0=qsel0[:], in1=qsel1[:], op=mybir.AluOpType.max
    )

    # ---- P1b: v' GEMMs (hide B1). vT layout: pos on partitions ----
    psctx = ExitStack()
    p1w = psctx.enter_context(tc.tile_pool(name="p1w", bufs=2, space="PSUM"))
    pmix = psctx.enter_context(tc.tile_pool(name="pmix", bufs=2, space="PSUM"))

    def emit_vprime(g):
        for pair in range(2):
            vps = p1w.tile([128, 2, C], F32, tag="vps")
            for i in range(2):
                blk = pair * 2 + i
                for kh in range(2):
                    nc.tensor.matmul(
                        vps[:, i, :], x_all[:, kh, g, ts(blk, 128)],
                        w2T[:, kh, :], start=(kh == 0), stop=(kh == 1),
                        skip_group_check=True,
                    )
            out_view = vbuf[:, :, g, ts(pair, 2 * 128)].rearrange(
                "p h (b c) -> p b h c", b=2
            )
            in_view = vps[:].rearrange("p b (h c) -> p b h c", h=2)
            nc.scalar.copy(out_view, in_view)

    def emit_mix(g):
        mix = pmix.tile([128, 2, FPT * Z], F32, tag="mix")
        for h in range(2):
            for blk in range(4):
                nc.tensor.matmul(
                    mix[:, h, ts(blk, 128)], vbuf[:, h, g, ts(blk, 128)],
                    w128[:], start=True, stop=True,
                )
        nc.scalar.copy(vbuf[:, :, g, :], mix[:])
        for h in range(2):
            nc.vector.bn_stats(stats[:, h, g, :], vbuf[:, h, g, :])

    VSPLIT = G
    for g in range(VSPLIT):
        emit_vprime(g)

    # ---- B1-dependent: offsets/attn -> M -> w128 (vector + psml) ----
    psml = psctx.enter_context(tc.tile_pool(name="psml", bufs=2, space="PSUM"))
    sa_ps = psml.tile([Z, 12], F32, tag="small")
    for kh in range(2):
        nc.tensor.matmul(
            sa_ps[:], qp16[:, kh, :], sawT[:, kh, :],
            start=(kh == 0), stop=(kh == 1),
        )
    logits = cp.tile([Z, 12], F32)
    nc.vector.tensor_add(logits[:], sa_ps[:], sab[:])
    off = cp.tile([Z, NHP], F32)
    nc.vector.tensor_scalar(
        out=off[:], in0=logits[:, 0:NHP], scalar1=0.0, scalar2=float(Z - 1),
        op0=mybir.AluOpType.max, op1=mybir.AluOpType.min,
    )
    ex = cp.tile([Z, NHP], F32)
    nc.scalar.activation(ex[:], logits[:, NHP:12], mybir.ActivationFunctionType.Exp)
    ssum = cp.tile([Z, 1], F32)
    nc.vector.tensor_reduce(
        out=ssum[:], in_=ex[:], axis=mybir.AxisListType.X, op=mybir.AluOpType.add
    )
    rinv = cp.tile([Z, 1], F32)
    nc.vector.reciprocal(rinv[:], ssum[:])
    attn = cp.tile([Z, NHP], F32)
    nc.vector.tensor_scalar_mul(attn[:], ex[:], rinv[:, 0:1])

    # M[z, y] = sum_p attn[z,p] * relu(1 - |off[z,p] - y|)
    msb = cp.tile([Z, Z], F32)
    mtmp = cp.tile([Z, Z], F32)
    dmy = cp.tile([Z, Z], F32)
    um = cp.tile([Z, Z], F32)
    vm = cp.tile([Z, Z], F32)
    for p in range(NHP):
        nc.vector.tensor_scalar(
            out=dmy[:], in0=iota[:], scalar1=off[:, p:p + 1], scalar2=None,
            op0=mybir.AluOpType.subtract,
        )
        nc.vector.tensor_scalar(
            out=um[:], in0=dmy[:], scalar1=-1.0, scalar2=1.0,
            op0=mybir.AluOpType.mult, op1=mybir.AluOpType.add,
        )
        nc.vector.tensor_scalar_add(vm[:], dmy[:], 1.0)
        nc.vector.tensor_tensor(
            out=um[:], in0=um[:], in1=vm[:], op=mybir.AluOpType.min
        )
        dst = msb if p == 0 else mtmp
        nc.vector.tensor_scalar(
            out=dst[:], in0=um[:], scalar1=0.0, scalar2=attn[:, p:p + 1],
            op0=mybir.AluOpType.max, op1=mybir.AluOpType.mult,
        )
        if p > 0:
            nc.vector.tensor_add(msb[:], msb[:], mtmp[:])
    m16 = cp.tile([Z, Z], F16)
    nc.vector.tensor_copy(m16[:], msb[:])
    mt_ps = psml.tile([Z, Z], F16, tag="small")
    nc.tensor.transpose(mt_ps[:], m16[:], ident[:])
    mt16 = cp.tile([Z, Z], F16)
    nc.vector.tensor_copy(mt16[:], mt_ps[:])
    for b4 in range(4):
        nc.vector.tensor_copy(w128[ts(b4, 32), ts(b4, 32)], mt16[:])

    # ---- P2a: remaining v' interleaved with mix; then mix tail ----
    for g in range(VSPLIT, G):
        emit_vprime(g)
        emit_mix(g - VSPLIT)
    for g in range(G - VSPLIT, G):
        emit_mix(g)

    # x16 (residual) reuses the buffer freed by xpe16 (same pool/tag)
    x16_all = xpe_pool.tile([128, 2, G, FPT * Z], F16, tag="xbig")
    rsrc = t["x16"].rearrange("(h p) f z -> p h f z", p=128)
    for j in range(NCH):
        nc.sync.dma_start(
            x16_all[:, :, ts(j, CH), :].rearrange("p h g (f z) -> p h (g f) z", z=Z),
            rsrc[:, :, ts(j, CH * FPT), :],
        )

    # ---- B2: global BN stats (8-core add) ----
    mv = cp.tile([128, 2, 2], F32)
    for h in range(2):
        nc.vector.bn_aggr(mv[:, h, :], stats[:, h, :, :])
    msq = cp.tile([128, 2, 1], F32)
    nc.vector.tensor_mul(msq[:], mv[:, :, 0:1], mv[:, :, 0:1])
    ex2 = cp.tile([128, 2, 1], F32)
    nc.vector.tensor_add(ex2[:], mv[:, :, 1:2], msq[:])
    red_in = cp.tile([128, 2, 2], F32)
    nc.vector.tensor_scalar_mul(red_in[:, :, 0:1], mv[:, :, 0:1], float(NLOC))
    nc.vector.tensor_scalar_mul(red_in[:, :, 1:2], ex2[:], float(NLOC))
    rin = dramp.tile([128, 2, 2], F32)
    rout = dramp.tile([128, 2, 2], F32)
    nc.sync.dma_start(rin[:], red_in[:])
    nc.gpsimd.collective_compute(
        "AllReduce", mybir.AluOpType.add, replica_groups=ALL_GROUP,
        ins=[rin[:].opt()], outs=[rout[:].opt()],
    )
    gst = cp.tile([128, 2, 2], F32)
    nc.gpsimd.dma_start(gst[:], rout[:])
    mean_g = cp.tile([128, 2, 1], F32)
    nc.vector.tensor_scalar_mul(mean_g[:], gst[:, :, 0:1], 1.0 / NTOT)
    ex2g = cp.tile([128, 2, 1], F32)
    nc.vector.tensor_scalar_mul(ex2g[:], gst[:, :, 1:2], 1.0 / NTOT)
    m2g = cp.tile([128, 2, 1], F32)
    nc.vector.tensor_mul(m2g[:], mean_g[:], mean_g[:])
    var_g = cp.tile([128, 2, 1], F32)
    nc.vector.tensor_sub(var_g[:], ex2g[:], m2g[:])
    eps_sb = cp.tile([128, 1], F32)
    nc.vector.memset(eps_sb[:], EPS)
    sd = cp.tile([128, 2, 1], F32)
    nc.scalar.activation(
        sd[:], var_g[:], mybir.ActivationFunctionType.Sqrt, bias=eps_sb[:]
    )
    rs = cp.tile([128, 2, 1], F32)
    nc.vector.reciprocal(rs[:], sd[:])
    s_sb = cp.tile([128, 2, 1], F32)
    nc.vector.tensor_mul(s_sb[:], rs[:], gam[:])
    ms_t = cp.tile([128, 2, 1], F32)
    nc.vector.tensor_mul(ms_t[:], mean_g[:], s_sb[:])
    t_sb = cp.tile([128, 2, 1], F32)
    nc.vector.tensor_sub(t_sb[:], bet[:], ms_t[:])

    psctx.close()

    # ---- P2b: normalize + residual + store in 4-tile chunks.
    # scalar: nrm0; vector: nrm1 + res1; gpsimd: res0.
    # out-DMA split across sync (h0) and scalar (h1) queues.
    out_v = t["out"].rearrange("(h p) f z -> p h f z", p=128)
    CF = CH * FPT * Z
    for j in range(NCH):
        nrm0 = resp.tile([128, CF], F16, tag="nrm0")
        nrm1 = resp.tile([128, CF], F16, tag="nrm1")
        nc.scalar.activation(
            nrm0[:], vbuf[:, 0, ts(j, CH), :],
            mybir.ActivationFunctionType.Identity,
            scale=s_sb[:, 0, :], bias=t_sb[:, 0, :],
        )
        nc.vector.tensor_scalar(
            out=nrm1[:], in0=vbuf[:, 1, ts(j, CH), :],
            scalar1=s_sb[:, 1, :], scalar2=t_sb[:, 1, :],
            op0=mybir.AluOpType.mult, op1=mybir.AluOpType.add,
        )
        res0 = resp.tile([128, CF], F16, tag="res0")
        res1 = resp.tile([128, CF], F16, tag="res1")
        nc.gpsimd.tensor_add(
            res0[:], nrm0[:],
            x16_all[:, 0, ts(j, CH), :].rearrange("p a b -> p (a b)"),
        )
        nc.vector.tensor_add(
            res1[:], nrm1[:],
            x16_all[:, 1, ts(j, CH), :].rearrange("p a b -> p (a b)"),
        )
        nc.sync.dma_start(
            out_v[:, 0, ts(j, CH * FPT), :],
            res0[:].rearrange("p (f z) -> p f z", z=Z),
        )
        nc.scalar.dma_start(
            out_v[:, 1, ts(j, CH * FPT), :],
            res1[:].rearrange("p (f z) -> p f z", z=Z),
        )
    ctx.close()


_BUILT = None


def _build():
    global _BUILT
    if _BUILT is not None:
        return _BUILT
    nc = bacc.Bacc("TRN2", target_bir_lowering=False, debug=False, num_devices=8)
    t = {}
    t["xpe16"] = nc.dram_tensor("xpe16", [C, FL, Z], F16, kind="ExternalInput").ap()
    t["x16"] = nc.dram_tensor("x16", [C, FL, Z], F16, kind="ExternalInput").ap()
    t["qwT16"] = nc.dram_tensor("qwT16", [C, C], F16, kind="ExternalInput").ap()
    t["vw"] = nc.dram_tensor("vw", [C, C], F32, kind="ExternalInput").ap()
    t["owT"] = nc.dram_tensor("owT", [C, C], F32, kind="ExternalInput").ap()
    t["sawT16"] = nc.dram_tensor("sawT16", [C, 12], F16, kind="ExternalInput").ap()
    t["sab"] = nc.dram_tensor("sab", [12], F32, kind="ExternalInput").ap()
    t["iota"] = nc.dram_tensor("iota", [Z, Z], F32, kind="ExternalInput").ap()
    t["ident"] = nc.dram_tensor("ident", [Z, Z], F16, kind="ExternalInput").ap()
    t["gb"] = nc.dram_tensor("gb", [C, 2], F32, kind="ExternalInput").ap()
    t["bsl"] = nc.dram_tensor("bsl", [128, 2], F32, kind="ExternalInput").ap()
    t["out"] = nc.dram_tensor("out", [C, FL, Z], F16, kind="ExternalOutput").ap()
    with tile.TileContext(nc) as tc:
        _emit(tc, t)
    nc.compile()
    _BUILT = nc
    return nc


def _make_pe():
    pos = np.arange(Z, dtype=np.float32)[:, None]
    div = np.exp(np.arange(0, C, 2, dtype=np.float32) * (-np.log(10000.0) / C))
    pe = np.zeros((Z, C), dtype=np.float32)
    pe[:, 0::2] = np.sin(pos * div)
    pe[:, 1::2] = np.cos(pos * div)
    return pe


def _prepare_in_maps(features, q_w, v_w, o_w, offs_w, offs_b, attn_w, attn_b,
                     gamma, beta):
    features = np.ascontiguousarray(np.asarray(features, dtype=np.float32))
    pe = _make_pe()  # (Z, C)
    shared = {
        "qwT16": np.ascontiguousarray(np.asarray(q_w).T).astype(np.float16),
        "vw": np.ascontiguousarray(np.asarray(v_w, dtype=np.float32)),
        "owT": np.ascontiguousarray(np.asarray(o_w).T.astype(np.float32)),
        "sawT16": np.concatenate(
            [np.asarray(offs_w).T, np.asarray(attn_w).T], axis=1
        ).astype(np.float16),
        "sab": np.concatenate(
            [np.asarray(offs_b), np.asarray(attn_b)]
        ).astype(np.float32),
        "iota": np.tile(np.arange(Z, dtype=np.float32)[None, :], (Z, 1)),
        "ident": np.eye(Z, dtype=np.float16),
        "gb": np.stack(
            [np.asarray(gamma, np.float32), np.asarray(beta, np.float32)], axis=1
        ),
    }
    in_maps = []
    for k in range(8):
        bi, hq = k // 4, k % 4
        xs = features[bi][:, :, hq * HL:(hq + 1) * HL, :]
        xs = np.ascontiguousarray(xs.transpose(0, 2, 3, 1)).reshape(C, FL, Z)
        xpe = xs + pe.T[:, None, :]  # pe[z, c] -> (C, 1, Z) broadcast
        m = dict(shared)
        m["x16"] = xs.astype(np.float16)
        m["xpe16"] = xpe.astype(np.float16)
        bsl = np.zeros((128, 2), np.float32)
        bsl[:, 1 - bi] = NEG
        m["bsl"] = bsl
        in_maps.append(m)
    return in_maps


def kernel(**inputs):
    nc = _build()
    in_maps = _prepare_in_maps(**inputs)
    res = bass_utils.run_bass_kernel_spmd(nc, in_maps, core_ids=list(range(8)))

    full = np.empty((B, C, Z, H, W), dtype=np.float32)
    for k in range(8):
        bi, hq = k // 4, k % 4
        o = res.results[k]["out"].astype(np.float32).reshape(C, HL, W, Z).transpose(0, 3, 1, 2)
        full[bi][:, :, hq * HL:(hq + 1) * HL, :] = o
    return full
